# revision 4
# baseline (speedup 1.0000x reference)
"""AlphaEntmax attention (alpha=1.5) on 8 Trainium2 NeuronCores — v2.

Sharding: batch*heads data-parallel, 4 heads per core (cores 0-3: batch 0,
cores 4-7: batch 1). fp16 operands everywhere on the matmul paths (4x PE
throughput vs fp32), fp32 PSUM/stats.

tau solver: 5 sweeps, prefix-subsample schedule [4,2,2,1,1], each sweep a
quadratic active-set update: with L=sum relu(xs+ntau), Q=sum relu^2,
k=#active over the sweep prefix and target fraction fr:
    delta = (L - sqrt(max(L^2 - k*(Q-fr), 0))) / k,  ntau -= delta
(exact solve of the local quadratic model; converges to |sum_p-1| ~ 1e-2,
validated end-to-end rel err ~6e-4 vs the 50-iter bisection reference,
no final normalization needed).

Stat passes per sweep (engine load-balanced ACT vs DVE):
  L: ACT Relu(bias=ntau, accum) writing u | DVE stt(add,max vs zeros, accum)
  Q: ACT Square(u, accum)                | DVE stt(add,mult vs u, accum)
  k: ACT Sign(u, accum)                  | DVE stt(mult 1e9, min vs ones, accum)
Final p is recomputed transposed (S^T via K=65 matmul folding ntau) as
relu(z)*z with ACT relu + DVE square, feeding p@v directly.
"""

from contextlib import ExitStack

import numpy as np

C = 2048
E = 1024
HLOC = 4
HS = 64
DH = HLOC * HS
NQT = C // 128
NEG = np.float32(-60000.0)   # fp16-representable mask value
SCHED = [4, 2, 2, 1, 1]      # per-sweep prefix divisor (all-quad)
ACT_MIN_W = 512              # ops at least this wide may go to ACT

_NC_CACHE = {}


def _build_nc():
    import concourse.bacc as bacc
    import concourse.mybir as mybir
    import concourse.tile as tile
    from concourse.masks import make_identity

    F32 = mybir.dt.float32
    F16 = mybir.dt.float16
    ALU = mybir.AluOpType
    AFT = mybir.ActivationFunctionType
    AX = mybir.AxisListType

    nc = bacc.Bacc("TRN2", target_bir_lowering=False, debug=False, num_devices=8)

    xt = nc.dram_tensor("xt", [E, C], F16, kind="ExternalInput")
    wq = nc.dram_tensor("wq", [E, DH], F16, kind="ExternalInput")
    wk = nc.dram_tensor("wk", [E, DH], F16, kind="ExternalInput")
    wv = nc.dram_tensor("wv", [E, DH], F16, kind="ExternalInput")
    bqd = nc.dram_tensor("bq", [1, DH], F16, kind="ExternalInput")
    bkd = nc.dram_tensor("bk", [1, DH], F16, kind="ExternalInput")
    bvd = nc.dram_tensor("bv", [1, DH], F16, kind="ExternalInput")
    wu = nc.dram_tensor("wu", [DH, E], F16, kind="ExternalInput")
    mbd = nc.dram_tensor("mb", [128, 128], F16, kind="ExternalInput")
    mbtd = nc.dram_tensor("mbt", [128, 128], F16, kind="ExternalInput")
    fr4d = nc.dram_tensor("fr4", [128, 16], F32, kind="ExternalInput")
    fr2d = nc.dram_tensor("fr2", [128, 16], F32, kind="ExternalInput")
    out = nc.dram_tensor("out", [C, E], F16, kind="ExternalOutput")

    # simple engine load balancer for elementwise passes
    eng_load = {"ACT": 0.0, "DVE": 0.0}

    def pick_engine(w, kind="stat"):
        if kind == "stat":            # accumulating stat pass, SBUF fp16 src
            ca = 0.833 * w + 572.0
            cd = 1.145 * w + 190.0
        elif kind == "psum":          # copy/relu from PSUM, no accum
            ca = 0.833 * w + 293.0
            cd = 1.19 * w + 110.0
        else:
            raise ValueError(kind)
        if kind == "stat" and w < ACT_MIN_W:
            eng_load["DVE"] += cd
            return "DVE"
        if eng_load["ACT"] + ca <= eng_load["DVE"] + cd:
            eng_load["ACT"] += ca
            return "ACT"
        eng_load["DVE"] += cd
        return "DVE"

    def charge(eng, ns):
        eng_load[eng] += ns

    with tile.TileContext(nc) as tc, ExitStack() as ctx:
        const = ctx.enter_context(tc.tile_pool(name="const", bufs=1))
        pers = ctx.enter_context(tc.tile_pool(name="pers", bufs=1))

        ident = const.tile([128, 128], F16, tag="ident", name="ident")
        make_identity(nc, ident[:])
        ident32 = const.tile([128, 128], F32, tag="ident32", name="ident32")
        make_identity(nc, ident32[:])
        ones_r = const.tile([1, 512], F16, tag="ones_r", name="ones_r")
        nc.vector.memset(ones_r[:], 1.0)
        zeros_w = const.tile([128, C], F16, tag="zeros_w", name="zeros_w")
        nc.vector.memset(zeros_w[:], 0.0)
        ones_w = const.tile([128, C], F16, tag="ones_w", name="ones_w")
        nc.vector.memset(ones_w[:], 1.0)
        mb = const.tile([128, 128], F16, tag="mb", name="mb")
        nc.sync.dma_start(mb[:], mbd[:])
        mbt = const.tile([128, 128], F16, tag="mbt", name="mbt")
        nc.sync.dma_start(mbt[:], mbtd[:])
        fr = {}
        for dv, frd in ((4, fr4d), (2, fr2d)):
            fr[dv] = const.tile([128, 16], F32, tag=f"fr{dv}", name=f"fr{dv}")
            nc.sync.dma_start(fr[dv][:], frd[:])
        wu_t = [const.tile([128, E], F16, tag=f"wu{i}", name=f"wu{i}")
                for i in range(2)]
        for i in range(2):
            nc.sync.dma_start(wu_t[i][:], wu[128 * i:128 * (i + 1), :])

        qT = [pers.tile([128, C], F16, tag=f"qT{i}", name=f"qT{i}") for i in range(2)]
        kT = [pers.tile([128, C], F16, tag=f"kT{i}", name=f"kT{i}") for i in range(2)]
        vt = [pers.tile([128, DH], F16, tag=f"vt{i}", name=f"vt{i}")
              for i in range(NQT)]
        ohd = [pers.tile([128, C], F16, tag=f"ohd{i}", name=f"ohd{i}")
               for i in range(2)]
        kaug = pers.tile([65, C], F16, tag="kaug", name="kaug")
        qaug = pers.tile([65, C], F16, tag="qaug", name="qaug")
        nc.vector.memset(kaug[64:65, :], 1.0)

        # ---- phase P: projections (fp16, 1 cyc/row) ----
        with ExitStack() as pctx:
            wpool = pctx.enter_context(tc.tile_pool(name="wpool", bufs=1))
            ppsum = pctx.enter_context(
                tc.tile_pool(name="ppsum", bufs=2, space="PSUM"))

            xt_t = [wpool.tile([128, C], F16, tag=f"xt{i}", name=f"xt{i}")
                    for i in range(8)]
            for i in range(8):
                nc.sync.dma_start(xt_t[i][:], xt[128 * i:128 * (i + 1), :])

            wtiles = {}
            btiles = {}
            for name, wd, bd in (("q", wq, bqd), ("k", wk, bkd), ("v", wv, bvd)):
                wtiles[name] = [wpool.tile([128, DH], F16, tag=f"w{name}{i}",
                                           name=f"w{name}{i}") for i in range(8)]
                for i in range(8):
                    nc.sync.dma_start(wtiles[name][i][:],
                                      wd[128 * i:128 * (i + 1), :])
                btiles[name] = wpool.tile([1, DH], F16, tag=f"b{name}",
                                          name=f"b{name}")
                nc.sync.dma_start(btiles[name][:], bd[:])

            # emit q/k for tile half 0 first so head 0/1 attention can start
            def emit_qk(half):
                hsl = slice(128 * half, 128 * (half + 1))
                for name, dst in (("q", qT), ("k", kT)):
                    for chk in range(C // 512):
                        csl = slice(512 * chk, 512 * (chk + 1))
                        ps = ppsum.tile([128, 512], F32, tag="pqk", name="pqk")
                        nc.tensor.matmul(ps[:], btiles[name][:, hsl], ones_r[:],
                                         start=True, stop=False)
                        for ei in range(8):
                            nc.tensor.matmul(ps[:], wtiles[name][ei][:, hsl],
                                             xt_t[ei][:, csl],
                                             start=False, stop=(ei == 7))
                        if pick_engine(512, "psum") == "ACT":
                            nc.scalar.activation(dst[half][:, csl], ps[:],
                                                 AFT.Copy)
                        else:
                            nc.vector.tensor_copy(dst[half][:, csl], ps[:])

            emit_qk(0)
            emit_qk(1)
            for ti in range(NQT):
                tsl = slice(128 * ti, 128 * (ti + 1))
                ps = ppsum.tile([128, DH], F32, tag="pv", name="pv")
                nc.tensor.matmul(ps[:], ones_r[:, :128], btiles["v"][:],
                                 start=True, stop=False)
                for ei in range(8):
                    nc.tensor.matmul(ps[:], xt_t[ei][:, tsl], wtiles["v"][ei][:],
                                     start=False, stop=(ei == 7))
                if pick_engine(DH, "psum") == "ACT":
                    nc.scalar.activation(vt[ti][:], ps[:], AFT.Copy)
                else:
                    nc.vector.tensor_copy(vt[ti][:], ps[:])

        # ---- phase A pools ----
        xs_pool = ctx.enter_context(tc.tile_pool(name="xs", bufs=NQT + 10))
        u_pool = ctx.enter_context(tc.tile_pool(name="u", bufs=4))
        ja_pool = ctx.enter_context(tc.tile_pool(name="ja", bufs=1))
        jd_pool = ctx.enter_context(tc.tile_pool(name="jd", bufs=1))
        pt_pool = ctx.enter_context(tc.tile_pool(name="pt", bufs=2))
        ptk_pool = ctx.enter_context(tc.tile_pool(name="ptk", bufs=4))
        ut_pool = ctx.enter_context(tc.tile_pool(name="ut", bufs=2))
        st_pool = ctx.enter_context(tc.tile_pool(name="st", bufs=3))
        sc_psum = ctx.enter_context(tc.tile_pool(name="scp", bufs=2, space="PSUM"))
        tr_psum = ctx.enter_context(tc.tile_pool(name="trp", bufs=1, space="PSUM"))
        pv_psum = ctx.enter_context(tc.tile_pool(name="pvp", bufs=2, space="PSUM"))

        junkA = ja_pool.tile([128, C], F16, tag="junkA", name="junkA")
        junkD = jd_pool.tile([128, C], F16, tag="junkD", name="junkD")

        def gen_scores(pair, state):
            """Generator: scores + copies + rowmax per qi; yields per qi."""
            qTh = qT[pair // 2][64 * (pair % 2):64 * (pair % 2) + 64, :]
            kTh = kT[pair // 2][64 * (pair % 2):64 * (pair % 2) + 64, :]
            mx = st_pool.tile([128, NQT], F32, tag="mx", name="mx")
            tau = st_pool.tile([128, NQT], F32, tag="tau", name="tau")
            state["tau"] = tau
            xs_g = {}
            state["xs"] = xs_g
            for qi in range(NQT):
                ncol = 128 * (qi + 1)
                xs = xs_pool.tile([128, C], F16, tag="xs", name="xs")
                xs_g[qi] = xs
                for c0 in range(0, ncol, 1024):
                    w = min(1024, ncol - c0)
                    ps = sc_psum.tile([128, 1024], F32, tag="sc", name="sc")
                    for s0 in range(0, w, 512):
                        sw = min(512, w - s0)
                        has_diag = (c0 + s0 + sw == ncol)
                        nc.tensor.matmul(ps[:, s0:s0 + sw],
                                         qTh[:, 128 * qi:128 * (qi + 1)],
                                         kTh[:, c0 + s0:c0 + s0 + sw],
                                         start=True, stop=not has_diag)
                        if has_diag:
                            nc.tensor.matmul(ps[:, s0 + sw - 128:s0 + sw],
                                             ident[:], mb[:],
                                             start=False, stop=True)
                    if pick_engine(w, "psum") == "ACT":
                        nc.scalar.activation(xs[:, c0:c0 + w], ps[:, :w],
                                             AFT.Copy)
                    else:
                        nc.vector.tensor_copy(xs[:, c0:c0 + w], ps[:, :w])
                # sampled rowmax (::4) -> tau0 lower bound
                nc.vector.reduce_max(mx[:, qi:qi + 1], xs[:, 0:ncol:4],
                                     axis=AX.X)
                charge("DVE", 1.3 * (ncol // 4) + 190)
                if qi == NQT - 1:
                    nc.vector.tensor_scalar(out=tau[:], in0=mx[:], scalar1=1.0,
                                            scalar2=-1.0, op0=ALU.subtract,
                                            op1=ALU.mult)
                yield

        def gen_sweeps(pair, state):
            """Generator: one solver sweep + update per yield."""
            tau = state["tau"]
            xs_g = state["xs"]
            Lt = st_pool.tile([128, NQT], F32, tag="Lt", name="Lt")
            Qt = st_pool.tile([128, NQT], F32, tag="Qt", name="Qt")
            Kt = st_pool.tile([128, NQT], F32, tag="Kt", name="Kt")
            d1 = st_pool.tile([128, NQT], F32, tag="d1", name="d1")
            d2 = st_pool.tile([128, NQT], F32, tag="d2", name="d2")
            for it, div in enumerate(SCHED):
                for qi in range(NQT):
                    w = max(128, (128 * (qi + 1)) // div)
                    u = u_pool.tile([128, C], F16, tag="u", name="u")
                    # L-pass (produces u)
                    if pick_engine(w) == "ACT":
                        nc.scalar.activation(u[:, :w], xs_g[qi][:, :w],
                                             AFT.Relu, bias=tau[:, qi:qi + 1],
                                             accum_out=Lt[:, qi:qi + 1])
                    else:
                        nc.vector.scalar_tensor_tensor(
                            out=u[:, :w], in0=xs_g[qi][:, :w],
                            scalar=tau[:, qi:qi + 1], in1=zeros_w[:, :w],
                            op0=ALU.add, op1=ALU.max,
                            accum_out=Lt[:, qi:qi + 1])
                    # Q-pass
                    if pick_engine(w) == "ACT":
                        nc.scalar.activation(junkA[:, :w], u[:, :w],
                                             AFT.Square,
                                             accum_out=Qt[:, qi:qi + 1])
                    else:
                        nc.vector.scalar_tensor_tensor(
                            out=junkD[:, :w], in0=xs_g[qi][:, :w],
                            scalar=tau[:, qi:qi + 1], in1=u[:, :w],
                            op0=ALU.add, op1=ALU.mult,
                            accum_out=Qt[:, qi:qi + 1])
                    # k-pass
                    if pick_engine(w) == "ACT":
                        nc.scalar.activation(junkA[:, :w], u[:, :w],
                                             AFT.Sign,
                                             accum_out=Kt[:, qi:qi + 1])
                    else:
                        nc.vector.scalar_tensor_tensor(
                            out=junkD[:, :w], in0=u[:, :w], scalar=1.0e9,
                            in1=ones_w[:, :w], op0=ALU.mult, op1=ALU.min,
                            accum_out=Kt[:, qi:qi + 1])
                # quad update: delta = (L - sqrt(max(L^2 - k(Q-fr),0)))/k
                if div > 1:
                    nc.vector.tensor_tensor(out=d1[:], in0=Qt[:],
                                            in1=fr[div][:], op=ALU.subtract)
                else:
                    nc.vector.tensor_scalar(out=d1[:], in0=Qt[:], scalar1=1.0,
                                            scalar2=None, op0=ALU.subtract)
                nc.vector.tensor_tensor(out=d1[:], in0=Kt[:], in1=d1[:],
                                        op=ALU.mult)
                nc.vector.tensor_tensor(out=d2[:], in0=Lt[:], in1=Lt[:],
                                        op=ALU.mult)
                nc.vector.tensor_tensor(out=d1[:], in0=d2[:], in1=d1[:],
                                        op=ALU.subtract)
                nc.vector.tensor_scalar(out=d1[:], in0=d1[:], scalar1=0.0,
                                        scalar2=None, op0=ALU.max)
                nc.scalar.activation(d1[:], d1[:], AFT.Sqrt)
                nc.vector.tensor_tensor(out=d1[:], in0=Lt[:], in1=d1[:],
                                        op=ALU.subtract)
                nc.vector.reciprocal(d2[:], Kt[:])
                nc.vector.tensor_tensor(out=d1[:], in0=d1[:], in1=d2[:],
                                        op=ALU.mult)
                nc.vector.tensor_tensor(out=tau[:], in0=tau[:], in1=d1[:],
                                        op=ALU.subtract)
                charge("DVE", 9 * 190)
                charge("ACT", 600)
                yield

        def gen_st(pair, state):
            """Generator: S^T recompute + p@v; yields per k-tile chunk."""
            tau = state["tau"]
            qTh = qT[pair // 2][64 * (pair % 2):64 * (pair % 2) + 64, :]
            kTh = kT[pair // 2][64 * (pair % 2):64 * (pair % 2) + 64, :]
            nc.vector.tensor_copy(kaug[0:64, :], kTh)
            nc.vector.tensor_copy(qaug[0:64, :], qTh)
            ntp = tr_psum.tile([128, 512], F32, tag="tr", name="tr")
            nc.tensor.transpose(ntp[:NQT, :128], tau[:, :NQT], ident32[:])
            ntT = pt_pool.tile([NQT, 128], F16, tag="ntT", name="ntT")
            nc.vector.tensor_copy(ntT[:], ntp[:NQT, :128])
            for qi in range(NQT):
                nc.sync.dma_start(qaug[64:65, 128 * qi:128 * (qi + 1)],
                                  ntT[qi:qi + 1, :])
            yield

            KTB = 4
            for ktb in range(NQT // KTB):
                ptks = {}
                for kt in range(KTB * ktb, KTB * (ktb + 1)):
                    q_lo = 128 * kt
                    ptk = ptk_pool.tile([128, C], F16, tag="ptk", name="ptk")
                    ptks[kt] = ptk
                    for c0 in range(max(1024 * (q_lo // 1024), 0), C, 1024):
                        if c0 + 1024 <= q_lo:
                            continue
                        w = 1024
                        ps = sc_psum.tile([128, 1024], F32, tag="sc", name="sc")
                        for s0 in range(0, w, 512):
                            a0 = c0 + s0
                            if a0 + 512 <= q_lo:
                                continue
                            b0 = max(a0, q_lo)
                            sw = a0 + 512 - b0
                            has_diag = (b0 == q_lo)
                            nc.tensor.matmul(ps[:, b0 - c0:b0 - c0 + sw],
                                             kaug[:, q_lo:q_lo + 128],
                                             qaug[:, b0:b0 + sw],
                                             start=True, stop=not has_diag)
                            if has_diag:
                                nc.tensor.matmul(ps[:, b0 - c0:b0 - c0 + 128],
                                                 ident[:], mbt[:],
                                                 start=False, stop=True)
                        b0 = max(c0, q_lo)
                        ww = c0 + 1024 - b0
                        # p^T chunk = relu(z)^2: either ACT relu + DVE square,
                        # or a single fused DVE ts(max0, pow2) from PSUM —
                        # pick whichever minimizes the projected makespan
                        ca = 0.833 * ww + 293.0      # ACT relu
                        cd_sq = 0.6 * ww + 190.0     # DVE square
                        cd_fused = 1.19 * ww + 110.0
                        opt_a = max(eng_load["ACT"] + ca,
                                    eng_load["DVE"] + cd_sq)
                        opt_b = max(eng_load["ACT"],
                                    eng_load["DVE"] + cd_fused + cd_sq)
                        if opt_a <= opt_b:
                            eng_load["ACT"] += ca
                            eng_load["DVE"] += cd_sq
                            ut = ut_pool.tile([128, 1024], F16, tag="ut",
                                              name="ut")
                            nc.scalar.activation(ut[:, :ww],
                                                 ps[:, b0 - c0:b0 - c0 + ww],
                                                 AFT.Relu)
                            nc.vector.tensor_tensor(out=ptk[:, b0:b0 + ww],
                                                    in0=ut[:, :ww],
                                                    in1=ut[:, :ww],
                                                    op=ALU.mult)
                        else:
                            eng_load["DVE"] += cd_fused + cd_sq
                            ut = ut_pool.tile([128, 1024], F16, tag="ut",
                                              name="ut")
                            nc.vector.tensor_scalar(
                                out=ut[:, :ww],
                                in0=ps[:, b0 - c0:b0 - c0 + ww],
                                scalar1=0.0, scalar2=None, op0=ALU.max)
                            nc.vector.tensor_tensor(out=ptk[:, b0:b0 + ww],
                                                    in0=ut[:, :ww],
                                                    in1=ut[:, :ww],
                                                    op=ALU.mult)
                    yield
                p2 = 64 * (pair % 2)
                od = ohd[pair // 2]
                for j in range(4):
                    kts = [kt for kt in range(KTB * ktb, KTB * (ktb + 1))
                           if 128 * kt < 512 * (j + 1)]
                    if not kts:
                        continue
                    a0 = max(512 * j, 128 * kts[0])
                    po = pv_psum.tile([128, 512], F32, tag="po", name="po",
                                      bufs=3)
                    for i, kt in enumerate(kts):
                        ak = max(512 * j, 128 * kt)
                        nc.tensor.matmul(
                            po[p2:p2 + 64, ak - 512 * j:512],
                            vt[kt][:, HS * pair:HS * (pair + 1)],
                            ptks[kt][:, ak:512 * (j + 1)],
                            start=(i == 0), stop=(i == len(kts) - 1))
                    if ktb == 0:
                        nc.vector.tensor_copy(
                            od[p2:p2 + 64, 512 * j:512 * (j + 1)],
                            po[p2:p2 + 64, :])
                    else:
                        nc.vector.tensor_tensor(
                            out=od[p2:p2 + 64, a0:512 * (j + 1)],
                            in0=od[p2:p2 + 64, a0:512 * (j + 1)],
                            in1=po[p2:p2 + 64, a0 - 512 * j:512], op=ALU.add)
                    charge("DVE", 700)
                yield

        # software pipeline across heads with interleaved emission:
        # while head p's solver sweeps run (DVE/ACT heavy), chunks of head
        # p+1's scores (PE+copies) and head p-1's S^T/p@v are emitted so
        # every engine keeps a mixed instruction stream.
        def advance(g, n):
            if g is None:
                return
            for _ in range(n):
                try:
                    next(g)
                except StopIteration:
                    return

        def drain(g):
            if g is None:
                return
            for _ in g:
                pass

        states = [dict() for _ in range(HLOC)]
        drain(gen_scores(0, states[0]))
        st_gens = {}
        for p in range(HLOC):
            g_next = (gen_scores(p + 1, states[p + 1])
                      if p + 1 < HLOC else None)
            g_prev = st_gens.pop(p - 1, None)
            for _ in gen_sweeps(p, states[p]):
                advance(g_next, 4)
                advance(g_prev, 4)
            drain(g_prev)
            st_gens[p] = gen_st(p, states[p])
        drain(st_gens.pop(HLOC - 1))

        # ---- phase O: output projection ----
        for qi in range(NQT):
            for ch in range(2):
                csl = slice(512 * ch, 512 * (ch + 1))
                ps = sc_psum.tile([128, 1024], F32, tag="sc", name="sc")
                for i in range(2):
                    nc.tensor.matmul(ps[:, :512],
                                     ohd[i][:, 128 * qi:128 * (qi + 1)],
                                     wu_t[i][:, csl], start=(i == 0),
                                     stop=(i == 1))
                osb = pt_pool.tile([128, 512], F16, tag="osb", name="osb")
                if pick_engine(512, "psum") == "ACT":
                    nc.scalar.activation(osb[:], ps[:, :512], AFT.Copy)
                else:
                    nc.vector.tensor_copy(osb[:], ps[:, :512])
                nc.sync.dma_start(out[128 * qi:128 * (qi + 1), csl], osb[:])

    nc.compile()
    return nc


def _get_nc():
    if "nc" not in _NC_CACHE:
        _NC_CACHE["nc"] = _build_nc()
    return _NC_CACHE["nc"]


def _entmax_bisect_np(X, alpha, n_iter=50):
    d = X.shape[-1]
    am1 = alpha - 1.0
    Xs = (X * am1).astype(np.float32)
    max_val = Xs.max(-1, keepdims=True)
    tau_lo = max_val - np.float32(1.0)
    tau_hi = max_val - np.float32((1.0 / d) ** (1.0 / am1))
    f_lo = (np.clip(Xs - tau_lo, 0, None) ** (1.0 / am1)).sum(-1, keepdims=True) - 1.0
    dm = tau_hi - tau_lo
    p_m = np.zeros_like(Xs)
    for _ in range(n_iter):
        dm = dm * 0.5
        tau_m = tau_lo + dm
        p_m = np.clip(Xs - tau_m, 0, None) ** (1.0 / am1)
        f_m = p_m.sum(-1, keepdims=True) - 1.0
        tau_lo = np.where(f_m * f_lo >= 0, tau_m, tau_lo)
    return p_m / p_m.sum(-1, keepdims=True)


def _numpy_fallback(x, mask, H, hs, alpha, Wq, bq, Wk, bk, Wv, bv, Wu, bu):
    b, c, e = x.shape
    q = (x @ Wq + bq).reshape(b, c, H, hs)
    k = (x @ Wk + bk).reshape(b, c, H, hs)
    v = (x @ Wv + bv).reshape(b, c, H, hs)
    dot = np.einsum('bqhd,bkhd->bhqk', q, k).astype(np.float32) / np.sqrt(hs)
    dot = np.where(mask[:, None], dot, -np.inf).astype(np.float32)
    p = _entmax_bisect_np(dot, float(alpha))
    o = np.einsum('bhqk,bkhd->bqhd', p, v).reshape(b, c, H * hs)
    return (o @ Wu + bu).astype(np.float32)


def make_in_maps(x, alpha_f, hs, Wq, bq, Wk, bk, Wv, bv, Wu):
    s = np.float32((alpha_f - 1.0) / np.sqrt(hs))
    mbias = np.triu(np.full((128, 128), NEG, np.float32), 1).astype(np.float16)
    mbiast = np.ascontiguousarray(mbias.T)
    r = np.arange(128)[:, None]
    qi = np.arange(16)[None, :]
    valid = 128 * qi + r + 1
    frs = {}
    for dv in (2, 4):
        w = np.maximum(128, (128 * (qi + 1)) // dv)
        frs[dv] = (np.minimum(w, valid) / valid).astype(np.float32)
    in_maps = []
    for core in range(8):
        bb = core // 4
        hsl = slice((core % 4) * DH, (core % 4) * DH + DH)
        in_maps.append({
            "xt": np.ascontiguousarray(x[bb].T).astype(np.float16),
            "wq": (np.ascontiguousarray(Wq[:, hsl]) * s).astype(np.float16),
            "bq": (bq[hsl] * s).reshape(1, DH).astype(np.float16),
            "wk": np.ascontiguousarray(Wk[:, hsl]).astype(np.float16),
            "bk": bk[hsl].reshape(1, DH).astype(np.float16),
            "wv": np.ascontiguousarray(Wv[:, hsl]).astype(np.float16),
            "bv": bv[hsl].reshape(1, DH).astype(np.float16),
            "wu": np.ascontiguousarray(Wu[hsl, :]).astype(np.float16),
            "mb": mbias,
            "mbt": mbiast,
            "fr4": frs[4],
            "fr2": frs[2],
        })
    return in_maps


def kernel(x, attention_mask, num_heads, head_size, alpha,
           Wq, bq, Wk, bk, Wv, bv, Wu, bu):
    x = np.asarray(x, np.float32)
    mask = np.asarray(attention_mask)
    H = int(num_heads)
    hs = int(head_size)
    alpha_f = float(np.asarray(alpha))
    Wq = np.asarray(Wq, np.float32); bq = np.asarray(bq, np.float32)
    Wk = np.asarray(Wk, np.float32); bk = np.asarray(bk, np.float32)
    Wv = np.asarray(Wv, np.float32); bv = np.asarray(bv, np.float32)
    Wu = np.asarray(Wu, np.float32); bu = np.asarray(bu, np.float32)
    b, c, e = x.shape

    causal = np.tril(np.ones((c, c), dtype=bool))
    supported = (
        (b, c, e, H, hs) == (2, C, E, 16, HS)
        and abs(alpha_f - 1.5) < 1e-6
        and all(np.array_equal(mask[i], causal) for i in range(b))
    )
    if not supported:
        return _numpy_fallback(x, mask, H, hs, alpha_f,
                               Wq, bq, Wk, bk, Wv, bv, Wu, bu)

    from concourse.bass_utils import run_bass_kernel_spmd

    nc = _get_nc()
    in_maps = make_in_maps(x, alpha_f, hs, Wq, bq, Wk, bk, Wv, bv, Wu)
    res = run_bass_kernel_spmd(nc, in_maps, core_ids=list(range(8)))
    o = [res.results[i]["out"].astype(np.float32) for i in range(8)]
    full = np.stack([o[0] + o[1] + o[2] + o[3],
                     o[4] + o[5] + o[6] + o[7]]) + bu
    return full.astype(np.float32)
